# revision 10
# baseline (speedup 1.0000x reference)
"""Batch graph attention kernel for Trainium2, SPMD over 8 NeuronCores.

Problem: nn_BatchGraphAttention (B=8, N=1024, D=512, H=8 heads, hd=64):
  q,k,v = x@W{q,k,v}.T ; attn = softmax(mask(q k^T / 8, adj)) ; out = attn v
  y = LayerNorm(out @ wo.T + bo + x) * gamma + beta

Sharding: data-parallel over batch B across the 8 cores (one batch element per
core, zero cross-core communication).

Per-core design (matmuls in float32r — full-rate fp32 on the PE, ~1e-4 rel):
 - host pre-transposes x -> xT (feature-major), pre-scales wq by 1/sqrt(hd)
 - qT,kT feature-major [D,N]; v token-major [N,D] with a ones-column appended
   per head (softmax denominators fall out of the same attn@v matmul)
 - per head: logits^T[m,n] = kT_h-chunk^T @ qT_h on PE, exp on ScalarE,
   mask-multiply by adj^T on VectorE (exp(-1e4)->0 === multiply by {0,1}),
   attn^T streams through PE against v_aug -> po [65,N], row 64 = denominators
 - normalize: fast reciprocal (DVE) + gpsimd partition-broadcast + DVE mul
 - out-projection consumed head-major (token-major output), residual +
   LayerNorm via bn_stats/bn_aggr and fused (y-mu)*rs tensor_scalar.
"""
import numpy as np
from contextlib import ExitStack

import concourse.bass as bass
import concourse.tile as tile
from concourse import bacc, mybir

B, N, D, H = 8, 1024, 512, 8
HD = D // H            # 64
P = 128
NB = N // P            # 8 row chunks
KC = D // P            # 4 feature chunks
VW = H * (HD + 1)      # 520 = v_aug width per m-chunk
EPS = 1e-5
N_CORES = 8
F32 = mybir.dt.float32
F32R = mybir.dt.float32r
AF = mybir.ActivationFunctionType
ALU = mybir.AluOpType


def build_nc(trivial_gb: bool, debug_outs: bool = False):
    nc = bacc.Bacc("TRN2", target_bir_lowering=False, debug=False,
                   num_devices=N_CORES)
    xT_d = nc.dram_tensor("xT", [P, KC * N], F32, kind="ExternalInput").ap()
    xpb_d = nc.dram_tensor("xpb", [P, NB * D], F32, kind="ExternalInput").ap()
    adj_d = nc.dram_tensor("adjT", [P, NB * N], F32, kind="ExternalInput").ap()
    wq_d = nc.dram_tensor("wqT", [P, KC * D], F32, kind="ExternalInput").ap()
    wk_d = nc.dram_tensor("wkT", [P, KC * D], F32, kind="ExternalInput").ap()
    wv_d = nc.dram_tensor("wvT", [P, KC * D], F32, kind="ExternalInput").ap()
    wo_d = nc.dram_tensor("woT", [HD, H * D], F32, kind="ExternalInput").ap()
    if not trivial_gb:
        gam_d = nc.dram_tensor("gam", [P, D], F32, kind="ExternalInput").ap()
        bet_d = nc.dram_tensor("bet", [P, D], F32, kind="ExternalInput").ap()
    out_d = nc.dram_tensor("out", [N, D], F32, kind="ExternalOutput").ap()
    dbg = {}
    if debug_outs:
        dbg["qT"] = nc.dram_tensor("dbg_qT", [P, KC * N], F32, kind="ExternalOutput").ap()
        dbg["kT"] = nc.dram_tensor("dbg_kT", [P, KC * N], F32, kind="ExternalOutput").ap()
        dbg["vA"] = nc.dram_tensor("dbg_vA", [P, NB * VW], F32, kind="ExternalOutput").ap()
        dbg["at0"] = nc.dram_tensor("dbg_at0", [P, N], F32, kind="ExternalOutput").ap()
        dbg["oT"] = nc.dram_tensor("dbg_oT", [HD, H * N], F32, kind="ExternalOutput").ap()

    with tile.TileContext(nc) as tc, ExitStack() as ctx:
        const = ctx.enter_context(tc.tile_pool(name="const", bufs=1))
        workx = ctx.enter_context(tc.tile_pool(name="workx", bufs=2))
        worka = ctx.enter_context(tc.tile_pool(name="worka", bufs=2))
        norm = ctx.enter_context(tc.tile_pool(name="norm", bufs=1))
        small = ctx.enter_context(tc.tile_pool(name="small", bufs=2))
        ppool = ctx.enter_context(tc.tile_pool(name="ppool", bufs=2, space="PSUM"))
        plpool = ctx.enter_context(tc.tile_pool(name="plpool", bufs=2, space="PSUM"))
        popool = ctx.enter_context(tc.tile_pool(name="popool", bufs=1, space="PSUM"))

        # ---- persistent tiles
        wqr = const.tile([P, KC * D], F32R)
        wkr = const.tile([P, KC * D], F32R)
        wvr = const.tile([P, KC * D], F32R)
        adj_sb = const.tile([P, NB * N], F32)     # adj^T fp32
        qT = const.tile([P, KC * N], F32R)
        kT = const.tile([P, KC * N], F32R)
        vA = const.tile([P, NB * VW], F32R)       # v token-major + ones cols
        ones32 = const.tile([P, NB * H], F32)
        zb = const.tile([P, 1], F32)      # zero bias for Exp
        eb = const.tile([P, 1], F32)      # eps bias for Sqrt
        nc.gpsimd.memset(zb[:], 0.0)
        nc.gpsimd.memset(eb[:], EPS)
        if not trivial_gb:
            gam_sb = const.tile([P, D], F32)
            bet_sb = const.tile([P, D], F32)

        with tc.tile_pool(name="proj", bufs=1) as proj, \
             tc.tile_pool(name="stage", bufs=2) as stage:
            xr = proj.tile([P, KC * N], F32R)

            cnt = [0]

            def load_round(dst, dram_ap, width, parts=P):
                for s in range(4):
                    q = width // 4
                    st = stage.tile([P, 1024], F32, tag="stage",
                                    name=f"stg{cnt[0]}")
                    cnt[0] += 1
                    nc.sync.dma_start(st[0:parts, 0:q],
                                      dram_ap[:, s * q:(s + 1) * q])
                    nc.vector.tensor_copy(dst[:, s * q:(s + 1) * q],
                                          st[0:parts, 0:q])

            load_round(wqr, wq_d, KC * D)
            load_round(wkr, wk_d, KC * D)
            load_round(xr, xT_d, KC * N)
            load_round(wvr, wv_d, KC * D)
            nc.sync.dma_start(adj_sb[:], adj_d[:])
            if not trivial_gb:
                nc.sync.dma_start(gam_sb[:], gam_d[:])
                nc.sync.dma_start(bet_sb[:], bet_d[:])

            # ---- projections: qT, kT feature-major
            for c in range(KC):
                for j in range(2):
                    pq = ppool.tile([P, 512], F32, tag="pp", name=f"pq{c}{j}")
                    for kc in range(KC):
                        nc.tensor.matmul(
                            pq[:],
                            wqr[:, kc * D + c * P: kc * D + (c + 1) * P],
                            xr[:, kc * N + j * 512: kc * N + (j + 1) * 512],
                            start=(kc == 0), stop=(kc == KC - 1))
                    nc.scalar.copy(qT[:, c * N + j * 512: c * N + (j + 1) * 512], pq[:])
                    pk = ppool.tile([P, 512], F32, tag="pp", name=f"pk{c}{j}")
                    for kc in range(KC):
                        nc.tensor.matmul(
                            pk[:],
                            wkr[:, kc * D + c * P: kc * D + (c + 1) * P],
                            xr[:, kc * N + j * 512: kc * N + (j + 1) * 512],
                            start=(kc == 0), stop=(kc == KC - 1))
                    nc.scalar.copy(kT[:, c * N + j * 512: c * N + (j + 1) * 512], pk[:])

            # ---- v token-major with interleaved ones columns
            nc.gpsimd.memset(ones32[:], 1.0)
            nc.vector.tensor_copy(
                vA[:].rearrange("p (m h s) -> p (m h) s", m=NB, h=H)[:, :, HD:HD + 1],
                ones32[:].rearrange("p (a b) -> p a b", b=1))
            for m in range(NB):
                pv = ppool.tile([P, 512], F32, tag="pp", name=f"pv{m}")
                for kc in range(KC):
                    nc.tensor.matmul(
                        pv[:],
                        xr[:, kc * N + m * P: kc * N + (m + 1) * P],
                        wvr[:, kc * D: (kc + 1) * D],
                        start=(kc == 0), stop=(kc == KC - 1))
                nc.scalar.copy(
                    vA[:, m * VW: (m + 1) * VW].rearrange("p (h s) -> p h s", h=H)[:, :, 0:HD],
                    pv[:].rearrange("p (h c) -> p h c", h=H))

        # late-lifetime tiles: allocated after the projection pool frees
        late = ctx.enter_context(tc.tile_pool(name="late", bufs=1))
        wor = late.tile([HD, H * D], F32R)        # head-major out-proj weights
        oT = late.tile([HD, H * N], F32R)         # attn out, head-major
        for s in range(4):
            q = H * D // 4
            st = workx.tile([P, N], F32, tag="exp", name=f"wostg{s}")
            nc.sync.dma_start(st[0:HD, 0:q], wo_d[:, s * q:(s + 1) * q])
            nc.vector.tensor_copy(wor[:, s * q:(s + 1) * q], st[0:HD, 0:q])

        if debug_outs:
            nc.sync.dma_start(dbg["qT"][:], qT[:].bitcast(F32))
            nc.sync.dma_start(dbg["kT"][:], kT[:].bitcast(F32))
            nc.sync.dma_start(dbg["vA"][:], vA[:].bitcast(F32))

        # ---- heads
        for c in range(KC):
            for half in range(2):
                h = 2 * c + half
                lo = HD * half
                po = popool.tile([HD + 1, N], F32, tag="po", name=f"po{h}")
                ats = []
                for i in range(NB):
                    pl = plpool.tile([P, N], F32, tag="pl", name=f"pl{h}_{i}")
                    for j in range(2):
                        nc.tensor.matmul(
                            pl[:, j * 512:(j + 1) * 512],
                            kT[lo:lo + HD, c * N + i * P: c * N + (i + 1) * P],
                            qT[lo:lo + HD, c * N + j * 512: c * N + (j + 1) * 512],
                            start=True, stop=True)
                    # drain previous chunk into po while this logits tile runs
                    if i > 0:
                        vsl = (i - 1) * VW + h * (HD + 1)
                        for j in range(2):
                            nc.tensor.matmul(
                                po[:, j * 512:(j + 1) * 512],
                                vA[:, vsl: vsl + HD + 1],
                                ats[i - 1][:, j * 512:(j + 1) * 512],
                                start=(i == 1), stop=False)
                    et = workx.tile([P, N], F32, tag="exp", name=f"et{h}_{i}")
                    nc.scalar.activation(et[:], pl[:], AF.Exp, bias=zb[:])
                    at = worka.tile([P, N], F32R, tag="attn", name=f"at{h}_{i}")
                    nc.vector.tensor_mul(at[:], et[:], adj_sb[:, i * N:(i + 1) * N])
                    ats.append(at)
                vsl = (NB - 1) * VW + h * (HD + 1)
                for j in range(2):
                    nc.tensor.matmul(
                        po[:, j * 512:(j + 1) * 512],
                        vA[:, vsl: vsl + HD + 1],
                        ats[NB - 1][:, j * 512:(j + 1) * 512],
                        start=False, stop=True)
                if debug_outs and h == 0:
                    nc.sync.dma_start(dbg["at0"][:], ats[0][:].bitcast(F32))
                # normalize: po row 64 (partition 64) holds denominators
                s64 = norm.tile([HD + 1, N], F32, tag="s64", name=f"s64_{h}")
                nc.scalar.copy(s64[HD:HD + 1, :], po[HD:HD + 1, :])
                s0 = norm.tile([1, N], F32, tag="s0", name=f"s0_{h}")
                nc.sync.dma_start(s0[0:1, :], s64[HD:HD + 1, :])
                rec = norm.tile([1, N], F32, tag="rec", name=f"rec{h}")
                nc.vector.reciprocal_approx_fast(rec[0:1, :], s0[0:1, :])
                rb = norm.tile([HD, N], F32, tag="rb", name=f"rb{h}")
                nc.gpsimd.partition_broadcast(rb[:], rec[0:1, :])
                nc.vector.tensor_mul(oT[:, h * N:(h + 1) * N], po[0:HD, :], rb[:])

        if debug_outs:
            nc.sync.dma_start(dbg["oT"][:], oT[:].bitcast(F32))

        # ---- out projection (token-major) + residual + LayerNorm
        for i in range(NB):
            pf = ppool.tile([P, 512], F32, tag="pp", name=f"pf{i}")
            for h in range(H):
                nc.tensor.matmul(
                    pf[:],
                    oT[:, h * N + i * P: h * N + (i + 1) * P],
                    wor[:, h * D:(h + 1) * D],
                    start=(h == 0), stop=(h == H - 1))
            xt = workx.tile([P, D], F32, tag="xpb", name=f"xt{i}")
            nc.sync.dma_start(xt[:], xpb_d[:, i * D:(i + 1) * D])
            y = workx.tile([P, D], F32, tag="y", name=f"y{i}")
            nc.vector.tensor_add(y[:], pf[:], xt[:])
            st6 = small.tile([P, 6], F32, tag="st6", name=f"st6_{i}")
            nc.vector.bn_stats(st6[:], y[:])
            mv = small.tile([P, 2], F32, tag="mv", name=f"mv{i}")
            nc.vector.bn_aggr(mv[:], st6[:])
            sd = small.tile([P, 1], F32, tag="sd", name=f"sd{i}")
            nc.scalar.activation(sd[:], mv[:, 1:2], AF.Sqrt, bias=eb[:])
            rs = small.tile([P, 1], F32, tag="rs", name=f"rs{i}")
            nc.vector.reciprocal(rs[:], sd[:])
            o_sb = workx.tile([P, D], F32, tag="osb", name=f"o{i}")
            nc.vector.tensor_scalar(o_sb[:], y[:], mv[:, 0:1], rs[:],
                                    op0=ALU.subtract, op1=ALU.mult)
            if not trivial_gb:
                nc.vector.tensor_mul(o_sb[:], o_sb[:], gam_sb[:])
                nc.vector.tensor_add(o_sb[:], o_sb[:], bet_sb[:])
            nc.sync.dma_start(out_d[i * P:(i + 1) * P, :], o_sb[:])

    nc.compile()
    return nc


def host_prep(x, adj, wq, wk, wv, wo, bo, gamma, beta):
    """Build per-core input maps (host-side numpy only)."""
    scale = np.float32(1.0 / np.sqrt(np.float32(HD)))
    wqT = np.ascontiguousarray(wq.T * scale, dtype=np.float32)   # [k, dq]
    wkT = np.ascontiguousarray(wk.T, dtype=np.float32)
    wvT = np.ascontiguousarray(wv.T, dtype=np.float32)
    woT = np.ascontiguousarray(wo.T, dtype=np.float32)           # [d, do]

    def chunk_pk(a, pdim=P):                       # [R, C] -> [pdim, (R//pdim)*C]
        rr, cc = a.shape
        return np.ascontiguousarray(
            a.reshape(rr // pdim, pdim, cc).transpose(1, 0, 2).reshape(pdim, -1))

    wq_c = chunk_pk(wqT)
    wk_c = chunk_pk(wkT)
    wv_c = chunk_pk(wvT)
    wo_hm = chunk_pk(woT, pdim=HD)                 # head-major [64, H*D]
    trivial_gb = bool(np.all(gamma == 1.0) and np.all(beta == 0.0))
    gam_t = np.ascontiguousarray(np.broadcast_to(gamma.astype(np.float32), (P, D)))
    bet_t = np.ascontiguousarray(np.broadcast_to(beta.astype(np.float32), (P, D)))

    xpb_all = (x + bo[None, None, :]).astype(np.float32)
    in_maps = []
    for b in range(B):
        xT = np.ascontiguousarray(x[b].T, dtype=np.float32)      # [D, N]
        adjT = np.ascontiguousarray(adj[b].T, dtype=np.float32)  # [N, N]
        m = {
            "xT": chunk_pk(xT),
            "xpb": chunk_pk(xpb_all[b]),
            "adjT": chunk_pk(adjT),
            "wqT": wq_c, "wkT": wk_c, "wvT": wv_c, "woT": wo_hm,
        }
        if not trivial_gb:
            m["gam"] = gam_t
            m["bet"] = bet_t
        in_maps.append(m)
    return in_maps, trivial_gb


# ---------------------------------------------------------------------------
# SPMD runner: persistent jitted callable (mirrors bass2jax.run_bass_via_pjrt
# but keeps the jit cache warm across calls).
# ---------------------------------------------------------------------------
class SpmdRunner:
    def __init__(self, nc, n_cores: int):
        import jax
        from jax.sharding import Mesh, PartitionSpec
        from jax.experimental.shard_map import shard_map
        from concourse.bass2jax import (_bass_exec_p, install_neuronx_cc_hook,
                                        partition_id_tensor)
        install_neuronx_cc_hook()
        self.jax = jax
        self.nc = nc
        self.n_cores = n_cores

        partition_name = (nc.partition_id_tensor.name
                          if nc.partition_id_tensor else None)
        in_names, out_names, out_avals, zero_outs = [], [], [], []
        for alloc in nc.m.functions[0].allocations:
            if not isinstance(alloc, mybir.MemoryLocationSet):
                continue
            name = alloc.memorylocations[0].name
            if alloc.kind == "ExternalInput":
                if name != partition_name:
                    in_names.append(name)
            elif alloc.kind == "ExternalOutput":
                shape = tuple(alloc.tensor_shape)
                dtype = mybir.dt.np(alloc.dtype)
                out_names.append(name)
                out_avals.append(jax.core.ShapedArray(shape, dtype))
                zero_outs.append(np.zeros(shape, dtype))
        self.in_names = in_names
        self.out_names = out_names
        self.out_avals = out_avals
        self.zero_outs = zero_outs
        n_params = len(in_names)
        n_outs = len(out_avals)
        all_in_names = list(in_names) + list(out_names)
        if partition_name is not None:
            all_in_names.append(partition_name)

        def _body(*args):
            operands = list(args)
            if partition_name is not None:
                operands.append(partition_id_tensor())
            outs = _bass_exec_p.bind(
                *operands,
                out_avals=tuple(out_avals),
                in_names=tuple(all_in_names),
                out_names=tuple(out_names),
                lowering_input_output_aliases=(),
                sim_require_finite=True,
                sim_require_nnan=True,
                nc=nc,
            )
            return tuple(outs)

        devices = jax.devices()[:n_cores]
        assert len(devices) == n_cores, (
            f"need {n_cores} cores, have {len(jax.devices())}")
        mesh = Mesh(np.asarray(devices), ("core",))
        in_specs = (PartitionSpec("core"),) * (n_params + n_outs)
        out_specs = (PartitionSpec("core"),) * n_outs
        donate = tuple(range(n_params, n_params + n_outs))
        self._fn = jax.jit(
            shard_map(_body, mesh=mesh, in_specs=in_specs,
                      out_specs=out_specs, check_rep=False),
            donate_argnums=donate, keep_unused=True)

    def __call__(self, in_maps):
        n = self.n_cores
        concat_in = [
            np.concatenate([np.asarray(in_maps[c][name]) for c in range(n)],
                           axis=0)
            for name in self.in_names
        ]
        concat_zeros = [np.zeros((n * z.shape[0], *z.shape[1:]), z.dtype)
                        for z in self.zero_outs]
        out_arrs = self._fn(*concat_in, *concat_zeros)
        self.jax.block_until_ready(out_arrs)
        return [
            {name: np.asarray(out_arrs[i]).reshape(n, *self.out_avals[i].shape)[c]
             for i, name in enumerate(self.out_names)}
            for c in range(n)
        ]


_CACHE = {}


def _get_runner(trivial_gb):
    if trivial_gb not in _CACHE:
        nc = build_nc(trivial_gb)
        _CACHE[trivial_gb] = SpmdRunner(nc, N_CORES)
    return _CACHE[trivial_gb]


def kernel(x, adj, wq, wk, wv, wo, bo, gamma, beta):
    x = np.asarray(x, dtype=np.float32)
    in_maps, trivial_gb = host_prep(
        x, np.asarray(adj, dtype=np.float32),
        np.asarray(wq, dtype=np.float32), np.asarray(wk, dtype=np.float32),
        np.asarray(wv, dtype=np.float32), np.asarray(wo, dtype=np.float32),
        np.asarray(bo, dtype=np.float32), np.asarray(gamma, dtype=np.float32),
        np.asarray(beta, dtype=np.float32))
    runner = _get_runner(trivial_gb)
    results = runner(in_maps)
    out = np.stack([results[b]["out"] for b in range(B)], axis=0)
    return out.astype(np.float32)
